# revision 1
# baseline (speedup 1.0000x reference)
"""Trainium2 distributed kernel for nn_CPAM_Module (CPAM attention block).

Math collapse (verified exact vs reference, ~2.6e-8 fro rel err in f64):
  te   = text_flat @ G_w.T + G_b                      (B, C)
  te_flat = te[:, :, None] * l  (rank-1 per batch)  =>
  proj_key / proj_value are rank-1 in n; energy[b,n,m] = s[b,n]*l[m] + const(n)
  softmax over m kills the const =>
  attn[b,n,m] = softmax_m(s[b,n] * l[m])
  s[b,n] = sum_c u[b,c] x[b,c,n] + b_q.kte[b],  u = kte @ W_q, kte = te @ W_k.T
  a[b,n] = (sum_j l_j e^{l_j s}) / (sum_j e^{l_j s})
  out    = gamma * (vte[b,c] * a[b,n] + b_v[c]) + x,  vte = te @ W_v.T

Sharding: contraction (TXT=153600) split 8 ways for the big G matmul;
ReduceScatter of te (bf16) hands each core its 32 batches; epilogue is
batch-parallel. x/out traffic is B-sharded (25.7 MB each per core).
"""

import sys

sys.path.insert(0, "/opt/trn_rl_repo")

import numpy as np
import ml_dtypes

from concourse import bass, bacc, mybir, tile
from concourse.bass_utils import run_bass_kernel_spmd

F32 = mybir.dt.float32
BF16 = mybir.dt.bfloat16
FP8 = mybir.dt.float8e4
GW_SCALE = 256.0
AF = mybir.ActivationFunctionType
ALU = mybir.AluOpType

N_CORES = 8
B, C, H, W = 256, 1024, 14, 14
N = H * W  # 196
C8 = 128
TXT = 150 * 1024
KSH = TXT // N_CORES  # 19200 txt-contraction shard per core
NK = KSH // 128  # 150 k-tiles
BL = B // N_CORES  # 32 local batches
CT = C // 128  # 8 c tiles
JT = 98  # j-tile (196 = 2*98)


def build(gamma: float, skip_gb: bool, skip_bq: bool, skip_bv: bool, single: bool = False, repeat: int = 1, loop_n: int = 0, part: str = 'all'):
    # single=True builds a 1-core variant with the ReduceScatter replaced by a
    # local DMA (same bytes landing in te_rs) so TimelineSim can model it.
    nc = bacc.Bacc(
        "TRN2",
        target_bir_lowering=False,
        debug=False,
        num_devices=1 if single else N_CORES,
    )

    text_t = nc.dram_tensor("text_t", [128, NK * B], FP8, kind="ExternalInput")
    g_wt = nc.dram_tensor("g_wt", [128, NK * C], FP8, kind="ExternalInput")
    xs = nc.dram_tensor("xs", [BL, 128, CT * N], BF16, kind="ExternalInput")
    w_vt = nc.dram_tensor("w_vt", [128, CT * C], BF16, kind="ExternalInput")
    w_kt = nc.dram_tensor("w_kt", [128, CT * C8], BF16, kind="ExternalInput")
    w_q = nc.dram_tensor("w_q", [C8, C], BF16, kind="ExternalInput")
    lrow = nc.dram_tensor("lrow", [1, N], F32, kind="ExternalInput")
    l_bc = nc.dram_tensor("l_bc", [128, N], BF16, kind="ExternalInput")
    lw = nc.dram_tensor("lw", [N, 2], BF16, kind="ExternalInput")
    g_b = nc.dram_tensor("g_b", [C8, CT], F32, kind="ExternalInput")
    b_q = nc.dram_tensor("b_q", [C8, 1], BF16, kind="ExternalInput")
    gbv = nc.dram_tensor("gbv", [C8, CT], F32, kind="ExternalInput")
    out = nc.dram_tensor("out", [BL, 128, CT * N], BF16, kind="ExternalOutput")

    with tile.TileContext(nc) as tc:
        with (
            tc.tile_pool(name="const", bufs=1) as const,
            tc.tile_pool(name="dram", bufs=1, space="DRAM") as dram,
        ):
            # Constants
            lbc_sb = const.tile([128, N], BF16, tag="lbc")
            nc.sync.dma_start(lbc_sb[:], l_bc[:, :])
            lw0 = const.tile([JT, 2], BF16, tag="lw0")
            lw1 = const.tile([JT, 2], BF16, tag="lw1")
            nc.sync.dma_start(lw0[:], lw[0:JT, :])
            nc.sync.dma_start(lw1[:], lw[JT : 2 * JT, :])
            wvt_sb = const.tile([128, CT, C], BF16, tag="wvt")
            nc.scalar.dma_start(wvt_sb[:].opt(), w_vt[:, :])
            wkt_sb = const.tile([128, CT, C8], BF16, tag="wkt")
            nc.scalar.dma_start(wkt_sb[:].opt(), w_kt[:, :])
            wq_sb = const.tile([C8, C], BF16, tag="wq")
            nc.sync.dma_start(wq_sb[:], w_q[:, :])
            if not skip_gb:
                gb_sb = const.tile([C8, CT], F32, tag="gb")
                nc.sync.dma_start(gb_sb[:], g_b[:, :])
            if not skip_bq:
                bq_sb = const.tile([C8, 1], BF16, tag="bq")
                nc.sync.dma_start(bq_sb[:], b_q[:, :])
            if not skip_bv:
                gbv_sb = const.tile([C8, CT], F32, tag="gbv")
                nc.sync.dma_start(gbv_sb[:], gbv[:, :])
            ones128 = const.tile([1, C8], F32, tag="ones128")
            nc.vector.memset(ones128[:], 1.0)

            te_full = dram.tile([B, C], BF16)
            te_rs = dram.tile([BL, C], BF16)

            if loop_n:
                assert single, "hardware loop timing mode is single-core only"
                loop_cm = tc.For_i(0, loop_n, 1)
                loop_cm.__enter__()
            for _rep in range(repeat):
                if part in ("all", "g"):
                    # ---- Phase 1: G matmul, te_partial[b, c] over local txt shard ----
                    with (
                        tc.tile_pool(name=f"gpsum{_rep}", bufs=4, space="PSUM") as gp,
                        tc.tile_pool(name=f"tl{_rep}", bufs=3) as tlp,
                        tc.tile_pool(name=f"gw{_rep}", bufs=3) as gwp,
                        tc.tile_pool(name=f"tesb{_rep}", bufs=4) as tesb,
                    ):
                        pt = [
                            [gp.tile([128, 512], F32, tag="gp", name=f"gp{_rep}_{m}{n2}") for n2 in range(2)]
                            for m in range(2)
                        ]
                        KB = 10  # k-tiles per DMA batch (150 = 15 * 10)
                        NPAIR = NK // 2
                        for g in range(NK // KB):
                            tl_t = tlp.tile([128, KB, B], FP8, tag="tl")
                            nc.sync.dma_start(tl_t[:].opt(), text_t[:, g * KB * B : (g + 1) * KB * B])
                            gw_t = gwp.tile([128, KB, C], FP8, tag="gw")
                            nc.sync.dma_start(gw_t[:].opt(), g_wt[:, g * KB * C : (g + 1) * KB * C])
                            for f in range(0, KB, 2):
                                j = (g * KB + f) // 2  # pair index
                                for m in range(2):
                                    for n2 in range(2):
                                        nc.tensor.matmul(
                                            pt[m][n2][:],
                                            tl_t[:, f : f + 2, m * 128 : (m + 1) * 128],
                                            gw_t[:, f : f + 2, n2 * 512 : (n2 + 1) * 512],
                                            start=(j == 0),
                                            stop=(j == NPAIR - 1),
                                            perf_mode=mybir.MatmulPerfMode.DoubleRow,
                                        )
                        for m in range(2):
                            for n2 in range(2):
                                ev = tesb.tile([128, 512], BF16, tag="tesb")
                                nc.scalar.mul(ev[:], pt[m][n2][:], 1.0 / GW_SCALE)
                                nc.sync.dma_start(
                                    te_full[m * 128 : (m + 1) * 128, n2 * 512 : (n2 + 1) * 512],
                                    ev[:],
                                )

                    # ---- Phase 2: ReduceScatter -> local te (32, 1024) bf16 ----
                    if single:
                        nc.sync.dma_start(te_rs[:, :], te_full[0:BL, :])
                    else:
                        nc.gpsimd.collective_compute(
                            "ReduceScatter",
                            ALU.add,
                            replica_groups=[list(range(N_CORES))],
                            ins=[te_full.opt()],
                            outs=[te_rs.opt()],
                        )

                if part in ("all", "epi"):
                    # ---- Phase 3: epilogue prep: teT, kteT, uT, gvteT ----
                    teT_sb = const.tile([128, CT, BL], BF16, tag="teT")
                    for t in range(CT):
                        nc.scalar.dma_start(
                            teT_sb[:, t, :],
                            te_rs.rearrange("b (t p) -> p t b", p=128)[:, t, :].opt(),
                        )
                    if not skip_gb:
                        for t in range(CT):
                            nc.vector.tensor_scalar_add(
                                teT_sb[:, t, :], teT_sb[:, t, :], gb_sb[:, t : t + 1]
                            )

                    uT_sb = const.tile([128, CT, BL], F32, tag="uT")
                    gvteT_sb = const.tile([128, CT, BL], F32, tag="gvteT")
                    bqd_row = const.tile([1, BL], F32, tag="bqd") if not skip_bq else None

                    with (
                        tc.tile_pool(name=f"ppsum{_rep}", bufs=2, space="PSUM") as pp,
                        tc.tile_pool(name=f"psmall{_rep}", bufs=2) as psm,
                    ):
                        # kteT (q, b) = sum_c W_kT[c, q] * teT[c, b]
                        kteT_ps = pp.tile([C8, BL], F32, tag="pp")
                        for t in range(CT):
                            nc.tensor.matmul(
                                kteT_ps[:],
                                wkt_sb[:, t, :],
                                teT_sb[:, t, :],
                                start=(t == 0),
                                stop=(t == CT - 1),
                            )
                        kteT_sb = psm.tile([C8, BL], BF16, tag="kteT")
                        nc.scalar.copy(kteT_sb[:], kteT_ps[:])

                        # uT (c, b) = sum_q W_q[q, c] * kteT[q, b]
                        for t in range(CT):
                            u_ps = pp.tile([128, BL], F32, tag="pp")
                            nc.tensor.matmul(
                                u_ps[:],
                                wq_sb[:, t * 128 : (t + 1) * 128],
                                kteT_sb[:],
                                start=True,
                                stop=True,
                            )
                            nc.scalar.copy(uT_sb[:, t, :], u_ps[:])

                        # bqdot[b] = sum_q kteT[q, b] * b_q[q]
                        if not skip_bq:
                            bq_ps = pp.tile([BL, 1], F32, tag="ppbq")
                            nc.tensor.matmul(bq_ps[:], kteT_sb[:], bq_sb[:], start=True, stop=True)
                            bqd_col = psm.tile([BL, 1], F32, tag="bqdc")
                            nc.scalar.copy(bqd_col[:], bq_ps[:])
                            nc.sync.dma_start(bqd_row[:].rearrange("o b -> o b 1"), bqd_col[:])

                        # gvteT (c', b) = gamma * sum_c W_vT[c, c'] * teT[c, b]
                        for mt in range(CT):
                            v_ps = pp.tile([128, BL], F32, tag="ppv")
                            for kt in range(CT):
                                nc.tensor.matmul(
                                    v_ps[:],
                                    wvt_sb[:, kt, mt * 128 : (mt + 1) * 128],
                                    teT_sb[:, kt, :],
                                    start=(kt == 0),
                                    stop=(kt == CT - 1),
                                )
                            nc.scalar.mul(gvteT_sb[:, mt, :], v_ps[:], float(gamma))

                    # ---- Phase 4: per-batch attention epilogue ----
                    # Wave-of-4 structure: all ACT Copy-class ops of a wave are
                    # emitted together, then the wave's Exps, then the previous
                    # wave's output Copies — ACT pays ~2 function-table switches
                    # per wave instead of 2 per batch.
                    with (
                        tc.tile_pool(name=f"xp{_rep}", bufs=32) as xp,
                        tc.tile_pool(name=f"op{_rep}", bufs=6) as op,
                        tc.tile_pool(name=f"esb{_rep}", bufs=4) as ep,
                        tc.tile_pool(name=f"small{_rep}", bufs=6) as sm,
                        tc.tile_pool(name=f"ps_z{_rep}", bufs=2, space="PSUM") as ps_z,
                        tc.tile_pool(name=f"ps_dn{_rep}", bufs=2, space="PSUM") as ps_dn,
                        tc.tile_pool(name=f"ps_ab{_rep}", bufs=2, space="PSUM") as ps_ab,
                    ):
                        WAVE = 4
                        st = {}

                        def front_a(b):
                            # x load + y[p,n] = sum_t uT[p,t,b] x[p,t,n]
                            xb = xp.tile([128, CT, N], BF16, tag="xb", name=f"xb{_rep}_{b}")
                            nc.gpsimd.dma_start(xb[:].opt(), xs[b].opt())
                            y_sb = sm.tile([128, N], BF16, tag="y")
                            nc.vector.tensor_scalar_mul(
                                y_sb[:], xb[:, 0, :], uT_sb[:, 0, b : b + 1]
                            )
                            for t in (1, 2, 3, 7):
                                nc.vector.scalar_tensor_tensor(
                                    y_sb[:], xb[:, t, :], uT_sb[:, t, b : b + 1], y_sb[:],
                                    ALU.mult, ALU.add,
                                )
                            tmp = sm.tile([128, 3, N], BF16, tag="ytmp", name=f"ytmp{_rep}_{b}")
                            for t in (4, 5, 6):
                                nc.scalar.activation(
                                    tmp[:, t - 4, :], xb[:, t, :], AF.Copy,
                                    scale=uT_sb[:, t, b : b + 1],
                                )
                            nc.gpsimd.tensor_add(tmp[:, 0, :], tmp[:, 0, :], tmp[:, 1, :])
                            nc.gpsimd.tensor_add(tmp[:, 0, :], tmp[:, 0, :], tmp[:, 2, :])
                            nc.vector.tensor_add(y_sb[:], y_sb[:], tmp[:, 0, :])
                            if not skip_bq:
                                nc.vector.tensor_scalar_add(
                                    y_sb[:, 0:1], y_sb[:, 0:1], bqd_row[0:1, b : b + 1]
                                )
                            st[b] = (xb, y_sb)

                        def front_b(b):
                            # Z = l_bcast^T.y ; E = exp(Z); den/num; a = num/den
                            xb, y_sb = st[b]
                            den_ps = ps_dn.tile([1, N], F32, tag="den")
                            num_ps = ps_dn.tile([1, N], F32, tag="num")
                            for jt in range(2):
                                z_ps = ps_z.tile([JT, N], F32, tag="z")
                                nc.tensor.matmul(
                                    z_ps[:],
                                    lbc_sb[:, jt * JT : (jt + 1) * JT],
                                    y_sb[:],
                                    start=True,
                                    stop=True,
                                )
                                e_sb = ep.tile([JT, N], BF16, tag="e")
                                nc.scalar.activation(e_sb[:], z_ps[:], AF.Exp)
                                lw_t = lw0 if jt == 0 else lw1
                                nc.tensor.matmul(
                                    den_ps[:], lw_t[:, 0:1], e_sb[:],
                                    start=(jt == 0), stop=(jt == 1),
                                )
                                nc.tensor.matmul(
                                    num_ps[:], lw_t[:, 1:2], e_sb[:],
                                    start=(jt == 0), stop=(jt == 1),
                                )
                            dinv = sm.tile([1, N], F32, tag="dinv")
                            nc.vector.reciprocal(dinv[:], den_ps[:])
                            a_sb = sm.tile([1, N], F32, tag="a")
                            nc.vector.tensor_mul(a_sb[:], num_ps[:], dinv[:])
                            st[b] = (xb, a_sb)

                        def back(b):
                            # out[c,n] = gvteT[c,b]*a[n] (+ g*b_v) + x[c,n]
                            xb, a_sb = st.pop(b)
                            ab_ps = ps_ab.tile([128, N], F32, tag="ab")
                            nc.tensor.matmul(
                                ab_ps[:], ones128[:], a_sb[:], start=True, stop=True
                            )
                            ob = op.tile([128, CT, N], BF16, tag="ob")
                            for t in range(5):
                                nc.vector.scalar_tensor_tensor(
                                    ob[:, t, :],
                                    ab_ps[:],
                                    gvteT_sb[:, t, b : b + 1],
                                    xb[:, t, :],
                                    ALU.mult,
                                    ALU.add,
                                )
                            for t in range(5, CT):
                                nc.scalar.activation(
                                    ob[:, t, :], ab_ps[:], AF.Copy,
                                    scale=gvteT_sb[:, t, b : b + 1],
                                )
                            nc.gpsimd.tensor_add(
                                ob[:, 5:CT, :], ob[:, 5:CT, :], xb[:, 5:CT, :]
                            )
                            if not skip_bv:
                                for t in range(CT):
                                    nc.vector.tensor_scalar_add(
                                        ob[:, t, :], ob[:, t, :], gbv_sb[:, t : t + 1]
                                    )
                            nc.sync.dma_start(out[b].opt(), ob[:].opt())

                        waves = [list(range(w, min(w + WAVE, BL))) for w in range(0, BL, WAVE)]
                        for wi, wave in enumerate(waves):
                            for b in wave:
                                front_a(b)
                            for b in wave:
                                front_b(b)
                            if wi >= 1:
                                for b in waves[wi - 1]:
                                    back(b)
                        for b in waves[-1]:
                            back(b)
            if loop_n:
                loop_cm.__exit__(None, None, None)

    nc.compile()
    return nc


def _prep_inputs(inputs):
    """Host-side sharding. Returns in_maps for the 8 cores."""
    x = np.ascontiguousarray(inputs["x"], dtype=np.float32).reshape(B, C, N)
    text = np.ascontiguousarray(inputs["text_embed"], dtype=np.float32).reshape(B, -1)
    G_w = np.asarray(inputs["G_w"], dtype=np.float32)
    l = np.asarray(inputs["l"], dtype=np.float32).reshape(1, N)
    W_q = np.asarray(inputs["W_q"], dtype=np.float32)
    W_k = np.asarray(inputs["W_k"], dtype=np.float32)
    W_v = np.asarray(inputs["W_v"], dtype=np.float32)
    b_v = np.asarray(inputs["b_v"], dtype=np.float32)
    b_q = np.asarray(inputs["b_q"], dtype=np.float32)
    G_b = np.asarray(inputs["G_b"], dtype=np.float32)
    gamma = float(np.asarray(inputs["gamma"]).reshape(-1)[0])

    bf = ml_dtypes.bfloat16
    f8 = ml_dtypes.float8_e4m3

    def pretile(a, p=128):
        # (T*p, F) -> (p, T*F): partition-major tiling for contiguous DMA
        tp, f = a.shape
        t = tp // p
        return np.ascontiguousarray(a.reshape(t, p, f).transpose(1, 0, 2).reshape(p, t * f))

    w_vt = pretile(np.ascontiguousarray(W_v.T).astype(bf))
    w_kt = pretile(np.ascontiguousarray(W_k.T).astype(bf))
    w_q = W_q.astype(bf)
    lw = np.stack([np.ones(N, np.float32), l[0]], axis=1)  # (196, 2)
    g_b_t = np.ascontiguousarray(G_b.reshape(CT, C8).T)  # (128, 8)
    gbv = np.ascontiguousarray((gamma * b_v).reshape(CT, C8).T)
    b_q_col = b_q.reshape(C8, 1).astype(bf)

    in_maps = []
    for i in range(N_CORES):
        sl = slice(i * KSH, (i + 1) * KSH)
        in_maps.append(
            {
                "text_t": pretile(np.ascontiguousarray(text[:, sl].T).astype(f8)),
                "g_wt": pretile((np.ascontiguousarray(G_w[:, sl].T) * 256.0).astype(f8)),
                "xs": np.ascontiguousarray(
                    x[i * BL : (i + 1) * BL]
                    .reshape(BL, CT, 128, N)
                    .transpose(0, 2, 1, 3)
                    .reshape(BL, 128, CT * N)
                ).astype(bf),
                "w_vt": w_vt,
                "w_kt": w_kt,
                "w_q": w_q,
                "lrow": l,
                "l_bc": np.ascontiguousarray(np.broadcast_to(l, (128, N))).astype(bf),
                "lw": lw.astype(bf),
                "g_b": g_b_t,
                "b_q": b_q_col,
                "gbv": gbv,
            }
        )
    meta = {
        "gamma": gamma,
        "skip_gb": not np.any(G_b),
        "skip_bq": not np.any(b_q),
        "skip_bv": not np.any(b_v),
    }
    return in_maps, meta


def _run(inputs, trace=False, repeat=1):
    in_maps, meta = _prep_inputs(inputs)
    nc = build(meta["gamma"], meta["skip_gb"], meta["skip_bq"], meta["skip_bv"], repeat=repeat)
    res = run_bass_kernel_spmd(nc, in_maps, core_ids=list(range(N_CORES)), trace=trace)
    outs = [
        res.results[i]["out"]
        .astype(np.float32)
        .reshape(BL, 128, CT, N)
        .transpose(0, 2, 1, 3)
        .reshape(BL, C, N)
        for i in range(N_CORES)
    ]
    full = np.concatenate(outs, axis=0).reshape(B, C, H, W)
    return full, res


def kernel(**inputs) -> np.ndarray:
    full, _ = _run(inputs, trace=False)
    return full


if __name__ == "__main__":
    import reference

    inputs = {k: np.asarray(v) for k, v in reference.setup_inputs().items()}
    got = kernel(**inputs)
    print("out shape:", got.shape, got.dtype)



# revision 30
# speedup vs baseline: 1.4682x; 1.4682x over previous
"""Trainium2 distributed kernel for nn_CPAM_Module (CPAM attention block).

Math collapse (verified exact vs reference, ~2.6e-8 fro rel err in f64):
  te   = text_flat @ G_w.T + G_b                      (B, C)
  te_flat = te[:, :, None] * l  (rank-1 per batch)  =>
  proj_key / proj_value are rank-1 in n; energy[b,n,m] = s[b,n]*l[m] + const(n)
  softmax over m kills the const =>
  attn[b,n,m] = softmax_m(s[b,n] * l[m])
  s[b,n] = sum_c u[b,c] x[b,c,n] + b_q.kte[b],  u = kte @ W_q, kte = te @ W_k.T
  a[b,n] = (sum_j l_j e^{l_j s}) / (sum_j e^{l_j s})
  out    = gamma * (vte[b,c] * a[b,n] + b_v[c]) + x,  vte = te @ W_v.T

Sharding: contraction (TXT=153600) split 8 ways for the big G matmul;
ReduceScatter of te (bf16) hands each core its 32 batches; epilogue is
batch-parallel. x/out traffic is B-sharded (25.7 MB each per core).

Epilogue structure (v1): per-batch work is pushed off the vector engines:
  - s[b,n] via PE (8 matmuls, u-column stationary, x tiles moving)
  - z = l (outer) s via PE rank-1 matmul, batches PAIRED (rhs [1, 392])
  - exp on Act only (no table switches)
  - num/den via PE ([98,1] stationary lw columns)
  - a = num/den on DVE; a-broadcast via PE rank-1 (ones stationary)
  - out tiles: 8 fused scalar_tensor_tensor per batch, split DVE/Pool
Three-pair-deep software pipeline keeps PE fed; 8 PSUM banks exactly.
"""

import sys

sys.path.insert(0, "/opt/trn_rl_repo")

import numpy as np
import ml_dtypes

from concourse import bass, bacc, mybir, tile
from concourse.bass_utils import run_bass_kernel_spmd

F32 = mybir.dt.float32
BF16 = mybir.dt.bfloat16
FP8 = mybir.dt.float8e4
GW_SCALE = 256.0
AF = mybir.ActivationFunctionType
ALU = mybir.AluOpType

N_CORES = 8
B, C, H, W = 256, 1024, 14, 14
N = H * W  # 196
BL = B // N_CORES  # 32 local batches
C8 = 128
TXT = 150 * 1024
KSH = TXT // N_CORES  # 19200 txt-contraction shard per core
NK = KSH // 128  # 150 k-tiles
CT = C // 128  # 8 c tiles
JT = 98  # j-tile (196 = 2*98)
NPAIR = BL // 2  # 16 batch pairs in the epilogue


def build(gamma: float, skip_gb: bool, skip_bq: bool, skip_bv: bool, single: bool = False, repeat: int = 1, loop_n: int = 0, part: str = 'all'):
    # single=True builds a 1-core variant with the ReduceScatter replaced by a
    # local DMA (same bytes landing in te_rs) so TimelineSim can model it.
    nc = bacc.Bacc(
        "TRN2",
        target_bir_lowering=False,
        debug=False,
        num_devices=1 if single else N_CORES,
    )

    text_t = nc.dram_tensor("text_t", [128, NK * B], FP8, kind="ExternalInput")
    g_wt = nc.dram_tensor("g_wt", [128, NK * C], FP8, kind="ExternalInput")
    xs = nc.dram_tensor("xs", [BL, 128, CT * N], BF16, kind="ExternalInput")
    w_vt = nc.dram_tensor("w_vt", [128, CT * C], BF16, kind="ExternalInput")
    w_kt = nc.dram_tensor("w_kt", [128, CT * C8], BF16, kind="ExternalInput")
    w_q = nc.dram_tensor("w_q", [C8, C], BF16, kind="ExternalInput")
    lrow_bf = nc.dram_tensor("lrow_bf", [1, N], BF16, kind="ExternalInput")
    lw = nc.dram_tensor("lw", [N, 2], BF16, kind="ExternalInput")
    eye32 = nc.dram_tensor("eye32", [32, 32], BF16, kind="ExternalInput")
    g_b = nc.dram_tensor("g_b", [C8, CT], F32, kind="ExternalInput")
    b_q = nc.dram_tensor("b_q", [C8, 1], BF16, kind="ExternalInput")
    gbv = nc.dram_tensor("gbv", [C8, CT], F32, kind="ExternalInput")
    out = nc.dram_tensor("out", [BL, 128, CT * N], BF16, kind="ExternalOutput")

    with tile.TileContext(nc) as tc:
        with (
            tc.tile_pool(name="const", bufs=1) as const,
            tc.tile_pool(name="dram", bufs=1, space="DRAM") as dram,
        ):
            # Constants
            lrow_sb = const.tile([1, N], BF16, tag="lrow")
            nc.sync.dma_start(lrow_sb[:], lrow_bf[:, :])
            lw0 = const.tile([JT, 2], BF16, tag="lw0")
            lw1 = const.tile([JT, 2], BF16, tag="lw1")
            nc.sync.dma_start(lw0[:], lw[0:JT, :])
            nc.sync.dma_start(lw1[:], lw[JT : 2 * JT, :])
            eye_sb = const.tile([32, 32], BF16, tag="eye")
            nc.sync.dma_start(eye_sb[:], eye32[:, :])
            wvt_sb = const.tile([128, CT, C], BF16, tag="wvt")
            nc.scalar.dma_start(wvt_sb[:].opt(), w_vt[:, :])
            wkt_sb = const.tile([128, CT, C8], BF16, tag="wkt")
            nc.scalar.dma_start(wkt_sb[:].opt(), w_kt[:, :])
            wq_sb = const.tile([C8, C], BF16, tag="wq")
            nc.sync.dma_start(wq_sb[:], w_q[:, :])
            if not skip_gb:
                gb_sb = const.tile([C8, CT], F32, tag="gb")
                nc.sync.dma_start(gb_sb[:], g_b[:, :])
            if not skip_bq:
                bq_sb = const.tile([C8, 1], BF16, tag="bq")
                nc.sync.dma_start(bq_sb[:], b_q[:, :])
            if not skip_bv:
                gbv_sb = const.tile([C8, CT], F32, tag="gbv")
                nc.sync.dma_start(gbv_sb[:], gbv[:, :])
            ones128 = const.tile([1, C8], BF16, tag="ones128")
            nc.vector.memset(ones128[:], 1.0)

            te_full = dram.tile([B, C], BF16)
            te_rs = dram.tile([BL, C], BF16)

            if loop_n:
                assert single, "hardware loop timing mode is single-core only"
                loop_cm = tc.For_i(0, loop_n, 1)
                loop_cm.__enter__()
            for _rep in range(repeat):
                # xb tiles live across phases: pool opened before phase 1 so
                # prefetch DMAs can be issued early (after g loads). All xb
                # loads ride the SP (sync) queue: in-order behind the g loads
                # and te writes, so they never steal phase-1 bandwidth, and
                # they stream during the ReduceScatter window.
                xp_cm = tc.tile_pool(name=f"xp{_rep}", bufs=34)
                xp = xp_cm.__enter__()
                xb_tiles = {}

                def load_xb(b):
                    xb = xp.tile([128, CT, N], BF16, tag="xb", name=f"xb{_rep}_{b}")
                    nc.sync.dma_start(xb[:].opt(), xs[b].opt())
                    xb_tiles[b] = xb

                if part in ("all", "g"):
                    # ---- Phase 1: G matmul, te_partial[b, c] over local txt shard ----
                    with (
                        tc.tile_pool(name=f"gpsum{_rep}", bufs=4, space="PSUM") as gp,
                        tc.tile_pool(name=f"tl{_rep}", bufs=3) as tlp,
                        tc.tile_pool(name=f"gw{_rep}", bufs=3) as gwp,
                        tc.tile_pool(name=f"tesb{_rep}", bufs=4) as tesb,
                    ):
                        pt = [
                            [gp.tile([128, 512], F32, tag="gp", name=f"gp{_rep}_{m}{n2}") for n2 in range(2)]
                            for m in range(2)
                        ]
                        KB = 10  # k-tiles per DMA batch (150 = 15 * 10)
                        NPAIRK = NK // 2
                        for g in range(NK // KB):
                            tl_t = tlp.tile([128, KB, B], FP8, tag="tl")
                            nc.sync.dma_start(tl_t[:].opt(), text_t[:, g * KB * B : (g + 1) * KB * B])
                            gw_t = gwp.tile([128, KB, C], FP8, tag="gw")
                            nc.sync.dma_start(gw_t[:].opt(), g_wt[:, g * KB * C : (g + 1) * KB * C])
                            for f in range(0, KB, 2):
                                j = (g * KB + f) // 2  # pair index
                                for m in range(2):
                                    for n2 in range(2):
                                        nc.tensor.matmul(
                                            pt[m][n2][:],
                                            tl_t[:, f : f + 2, m * 128 : (m + 1) * 128],
                                            gw_t[:, f : f + 2, n2 * 512 : (n2 + 1) * 512],
                                            start=(j == 0),
                                            stop=(j == NPAIRK - 1),
                                            perf_mode=mybir.MatmulPerfMode.DoubleRow,
                                        )
                        for m in range(2):
                            for n2 in range(2):
                                ev = tesb.tile([128, 512], BF16, tag="tesb")
                                nc.scalar.mul(ev[:], pt[m][n2][:], 1.0 / GW_SCALE)
                                nc.sync.dma_start(
                                    te_full[m * 128 : (m + 1) * 128, n2 * 512 : (n2 + 1) * 512],
                                    ev[:],
                                )

                    # ---- Phase 2: ReduceScatter -> local te (32, 1024) bf16 ----
                    if single:
                        nc.sync.dma_start(te_rs[:, :], te_full[0:BL, :])
                    else:
                        nc.gpsimd.collective_compute(
                            "ReduceScatter",
                            ALU.add,
                            replica_groups=[list(range(N_CORES))],
                            ins=[te_full.opt()],
                            outs=[te_rs.opt()],
                        )
                    # prefetch the first x batches; in-order on SP behind the
                    # te writes, these fill the DMA during the RS window
                    for b in range(8):
                        load_xb(b)

                if part in ("all", "epi"):
                    # ---- Phase 3: prep: teT (PE transpose), kteT, uT, gvteT ----
                    # te_sb load rides the Act queue so it is not stuck behind
                    # the xb prefetch stream on SP
                    te_sb = const.tile([BL, C], BF16, tag="te_sb")
                    nc.scalar.dma_start(te_sb[:], te_rs[:, :])
                    teT_bf = const.tile([128, CT, BL], BF16, tag="teT")
                    uT_bf = const.tile([128, CT, BL], BF16, tag="uT")
                    gvteT = const.tile([128, CT, BL], F32, tag="gvteT")
                    gvte_sb = const.tile([BL, C], BF16, tag="gvte_sb")
                    bqd_row = const.tile([1, BL], F32, tag="bqd") if not skip_bq else None

                    with (
                        tc.tile_pool(name=f"ppsum{_rep}", bufs=1, space="PSUM") as pp,
                        tc.tile_pool(name=f"ppsum2{_rep}", bufs=1, space="PSUM") as pp2,
                        tc.tile_pool(name=f"psmall{_rep}", bufs=2) as psm,
                    ):
                        # teT[c, b] via PE transpose of te slices
                        tp_ps = pp.tile([128, CT, BL], BF16, tag="tp")
                        for t in range(CT):
                            nc.tensor.transpose(
                                tp_ps[:, t, :],
                                te_sb[:, t * 128 : (t + 1) * 128],
                                eye_sb[:],
                            )
                        if skip_gb:
                            nc.vector.tensor_scalar_mul(teT_bf[:], tp_ps[:], 1.0)
                        else:
                            for t in range(CT):
                                nc.vector.tensor_scalar_add(
                                    teT_bf[:, t, :], tp_ps[:, t, :], gb_sb[:, t : t + 1]
                                )

                        # kteT (q, b) = sum_c W_kT[c, q] * teT[c, b]
                        kteT_ps = pp.tile([C8, BL], F32, tag="kteT_ps")
                        for t in range(CT):
                            nc.tensor.matmul(
                                kteT_ps[:],
                                wkt_sb[:, t, :],
                                teT_bf[:, t, :],
                                start=(t == 0),
                                stop=(t == CT - 1),
                            )
                        kteT_sb = psm.tile([C8, BL], BF16, tag="kteT")
                        nc.vector.tensor_scalar_mul(kteT_sb[:], kteT_ps[:], 1.0)

                        # uT (c, b) = sum_q W_q[q, c] * kteT[q, b]
                        uT_ps = pp.tile([128, CT, BL], F32, tag="uT_ps")
                        for t in range(CT):
                            nc.tensor.matmul(
                                uT_ps[:, t, :],
                                wq_sb[:, t * 128 : (t + 1) * 128],
                                kteT_sb[:],
                                start=True,
                                stop=True,
                            )
                        nc.vector.tensor_scalar_mul(uT_bf[:], uT_ps[:], 1.0)

                        # bqdot[b] = sum_q kteT[q, b] * b_q[q]
                        if not skip_bq:
                            bq_ps = pp.tile([BL, 1], F32, tag="ppbq")
                            nc.tensor.matmul(bq_ps[:], kteT_sb[:], bq_sb[:], start=True, stop=True)
                            bqd_col = psm.tile([BL, 1], F32, tag="bqdc")
                            nc.scalar.copy(bqd_col[:], bq_ps[:])
                            nc.sync.dma_start(bqd_row[:].rearrange("o b -> o b 1"), bqd_col[:])

                        # gvte (b, c') = gamma * sum_c te[b, c] W_vT[c, c']
                        # computed b-major (16 wide matmuls), then PE-transposed
                        gv_ps = [
                            pp2.tile([BL, 512], F32, tag=f"gv{h}", name=f"gv{_rep}_{h}")
                            for h in range(2)
                        ]
                        for h in range(2):
                            for kt in range(CT):
                                nc.tensor.matmul(
                                    gv_ps[h][:],
                                    teT_bf[:, kt, :],
                                    wvt_sb[:, kt, h * 512 : (h + 1) * 512],
                                    start=(kt == 0),
                                    stop=(kt == CT - 1),
                                )
                            nc.vector.tensor_scalar_mul(
                                gvte_sb[:, h * 512 : (h + 1) * 512], gv_ps[h][:], 1.0
                            )
                        tp2_ps = pp.tile([128, CT, BL], BF16, tag="tp2")
                        for t in range(CT):
                            nc.tensor.transpose(
                                tp2_ps[:, t, :],
                                gvte_sb[:, t * 128 : (t + 1) * 128],
                                eye_sb[:],
                            )
                        nc.vector.tensor_scalar_mul(gvteT[:], tp2_ps[:], float(gamma))

                    # Emit the bulk of the xb prefetch BEFORE the phase-4 PSUM
                    # pools open: pool-open waits on the prep pools' release,
                    # and that wait would stall the in-order SP queue (and
                    # with it the prefetch stream) for ~5us.
                    next_load = 8
                    while next_load < 26:
                        load_xb(next_load)
                        next_load += 1

                    # ---- Phase 4: per-pair attention epilogue, 5-deep pipeline ----
                    with (
                        tc.tile_pool(name=f"op{_rep}", bufs=6) as op,
                        tc.tile_pool(name=f"esb{_rep}", bufs=4) as ep,
                        tc.tile_pool(name=f"small{_rep}", bufs=8) as sm,
                        tc.tile_pool(name=f"ps_s{_rep}", bufs=2, space="PSUM") as ps_s,
                        tc.tile_pool(name=f"ps_z{_rep}", bufs=2, space="PSUM") as ps_z,
                        tc.tile_pool(name=f"ps_nd{_rep}", bufs=2, space="PSUM") as ps_nd,
                        tc.tile_pool(name=f"ps_ab{_rep}", bufs=2, space="PSUM") as ps_ab,
                    ):
                        st = {}

                        def stage_y(p):
                            # s[b, n] = sum_c u[b, c] x[b, c, n] on PE; pair in
                            # one PSUM tile. PE only — the SBUF copy is its own
                            # (later) stage so PE never waits on Act.
                            s_ps = ps_s.tile([1, 2, N], F32, tag="s")
                            for i in range(2):
                                b = 2 * p + i
                                xb = xb_tiles[b]
                                for t in range(CT):
                                    nc.tensor.matmul(
                                        s_ps[:, i, :],
                                        uT_bf[:, t, b : b + 1],
                                        xb[:, t, :],
                                        start=(t == 0),
                                        stop=(t == CT - 1),
                                    )
                            st[p] = {"s_ps": s_ps}

                        def stage_scopy(p):
                            s_ps = st[p].pop("s_ps")
                            s_sb = sm.tile([1, 2, N], BF16, tag="s_sb")
                            nc.scalar.copy(s_sb[:], s_ps[:])
                            if not skip_bq:
                                for i in range(2):
                                    b = 2 * p + i
                                    nc.vector.tensor_scalar_add(
                                        s_sb[:, i, :], s_sb[:, i, :], bqd_row[0:1, b : b + 1]
                                    )
                            st[p]["s_sb"] = s_sb

                        def stage_z(p):
                            # z = l (outer) s, both jt halves; exp on Act.
                            s_sb = st[p]["s_sb"]
                            es = []
                            for jt in range(2):
                                z_ps = ps_z.tile([JT, 2 * N], F32, tag="z")
                                nc.tensor.matmul(
                                    z_ps[:],
                                    lrow_sb[0:1, jt * JT : (jt + 1) * JT],
                                    s_sb[:].rearrange("o i n -> o (i n)"),
                                    start=True,
                                    stop=True,
                                )
                                e_sb = ep.tile([JT, 2, N], BF16, tag="e")
                                nc.scalar.activation(
                                    e_sb[:].rearrange("j i n -> j (i n)"), z_ps[:], AF.Exp
                                )
                                es.append(e_sb)
                            st[p]["es"] = es

                        def stage_nd(p):
                            # den/num rows for the whole pair at once:
                            # [98,1] stationary lw col x e[98, 392] -> [1, 392]
                            es = st[p]["es"]
                            den_ps = ps_nd.tile([1, 2, N], F32, tag="nd", name=f"den{_rep}_{p}")
                            num_ps = ps_nd.tile([1, 2, N], F32, tag="nd", name=f"num{_rep}_{p}")
                            for col, nd_ps in ((0, den_ps), (1, num_ps)):
                                for jt in range(2):
                                    lw_t = lw0 if jt == 0 else lw1
                                    nc.tensor.matmul(
                                        nd_ps[:].rearrange("o i n -> o (i n)"),
                                        lw_t[:, col : col + 1],
                                        es[jt][:].rearrange("j i n -> j (i n)"),
                                        start=(jt == 0),
                                        stop=(jt == 1),
                                    )
                            a_sb = sm.tile([1, 2, N], BF16, tag="a")
                            dinv = sm.tile([1, 2, N], F32, tag="dinv")
                            nc.vector.reciprocal(dinv[:], den_ps[:])
                            nc.vector.tensor_mul(a_sb[:], num_ps[:], dinv[:])
                            st[p]["a_sb"] = a_sb

                        def stage_back(p):
                            # ab = ones (outer) a; evac to SBUF bf16 on Act.
                            # out tiles = ab*gvte + x. GPSIMD (Pool) has no
                            # PSUM port and supports only TensorTensor, so the
                            # Pool tiles get a pre-scaled copy (DVE 4x ts-mult
                            # or Act copy-with-scale) followed by a Pool add.
                            d = st.pop(p)
                            a_sb = d["a_sb"]
                            ab_ps = ps_ab.tile([128, 2 * N], F32, tag="ab")
                            nc.tensor.matmul(
                                ab_ps[:],
                                ones128[:],
                                a_sb[:].rearrange("o i n -> o (i n)"),
                                start=True,
                                stop=True,
                            )
                            ab_sb = sm.tile([128, 2, N], BF16, tag="ab_sb")
                            nc.scalar.copy(ab_sb[:].rearrange("c i n -> c (i n)"), ab_ps[:])
                            for i in range(2):
                                b = 2 * p + i
                                xb = xb_tiles.pop(b)
                                ob = op.tile([128, CT, N], BF16, tag="ob")
                                tmp = sm.tile([128, 4, N], BF16, tag="tmp")
                                # fused DVE tiles (4; plus t=7 on even batch)
                                stt_tiles = (0, 1, 2, 3, 7) if i == 0 else (0, 1, 2, 3)
                                for t in stt_tiles:
                                    nc.vector.scalar_tensor_tensor(
                                        ob[:, t, :], ab_sb[:, i, :],
                                        gvteT[:, t, b : b + 1], xb[:, t, :],
                                        ALU.mult, ALU.add,
                                    )
                                # DVE 4x-mode scaled copies for Pool adds
                                nc.vector.tensor_scalar_mul(
                                    tmp[:, 0, :], ab_sb[:, i, :], gvteT[:, 4, b : b + 1]
                                )
                                nc.vector.tensor_scalar_mul(
                                    tmp[:, 1, :], ab_sb[:, i, :], gvteT[:, 5, b : b + 1]
                                )
                                # Act copy-with-scale for the rest
                                act_tiles = (6,) if i == 0 else (6, 7)
                                for k, t in enumerate(act_tiles):
                                    nc.scalar.activation(
                                        tmp[:, 2 + k, :], ab_sb[:, i, :], AF.Copy,
                                        scale=gvteT[:, t, b : b + 1],
                                    )
                                pool_tiles = (4, 5) + act_tiles
                                for k, t in enumerate(pool_tiles):
                                    nc.gpsimd.tensor_add(
                                        ob[:, t, :], tmp[:, k, :], xb[:, t, :]
                                    )
                                if not skip_bv:
                                    for t in range(CT):
                                        nc.vector.tensor_scalar_add(
                                            ob[:, t, :], ob[:, t, :], gbv_sb[:, t : t + 1]
                                        )
                                nc.sync.dma_start(out[b].opt(), ob[:].opt())

                        # software pipeline, 5 deep so every PE op's inputs are
                        # a full iteration old (PE streams without waiting):
                        #   y(p) | scopy(p-1) | z(p-2) | nd(p-3) | back(p-4)
                        LOOKAHEAD = 8  # pairs of xb loads beyond current stage_y
                        for p in range(NPAIR + 4):
                            while next_load < BL and next_load < 2 * (p + 1 + LOOKAHEAD):
                                load_xb(next_load)
                                next_load += 1
                            if p < NPAIR:
                                stage_y(p)
                            if 0 <= p - 1 < NPAIR:
                                stage_scopy(p - 1)
                            if 0 <= p - 2 < NPAIR:
                                stage_z(p - 2)
                            if 0 <= p - 3 < NPAIR:
                                stage_nd(p - 3)
                            if 0 <= p - 4 < NPAIR:
                                stage_back(p - 4)
                xp_cm.__exit__(None, None, None)
            if loop_n:
                loop_cm.__exit__(None, None, None)

    nc.compile()
    return nc


def _prep_inputs(inputs):
    """Host-side sharding. Returns in_maps for the 8 cores."""
    x = np.ascontiguousarray(inputs["x"], dtype=np.float32).reshape(B, C, N)
    text = np.ascontiguousarray(inputs["text_embed"], dtype=np.float32).reshape(B, -1)
    G_w = np.asarray(inputs["G_w"], dtype=np.float32)
    l = np.asarray(inputs["l"], dtype=np.float32).reshape(1, N)
    W_q = np.asarray(inputs["W_q"], dtype=np.float32)
    W_k = np.asarray(inputs["W_k"], dtype=np.float32)
    W_v = np.asarray(inputs["W_v"], dtype=np.float32)
    b_v = np.asarray(inputs["b_v"], dtype=np.float32)
    b_q = np.asarray(inputs["b_q"], dtype=np.float32)
    G_b = np.asarray(inputs["G_b"], dtype=np.float32)
    gamma = float(np.asarray(inputs["gamma"]).reshape(-1)[0])

    bf = ml_dtypes.bfloat16
    f8 = ml_dtypes.float8_e4m3

    def pretile(a, p=128):
        # (T*p, F) -> (p, T*F): partition-major tiling for contiguous DMA
        tp, f = a.shape
        t = tp // p
        return np.ascontiguousarray(a.reshape(t, p, f).transpose(1, 0, 2).reshape(p, t * f))

    w_vt = pretile(np.ascontiguousarray(W_v.T).astype(bf))
    w_kt = pretile(np.ascontiguousarray(W_k.T).astype(bf))
    w_q = W_q.astype(bf)
    lw = np.stack([np.ones(N, np.float32), l[0]], axis=1)  # (196, 2)
    g_b_t = np.ascontiguousarray(G_b.reshape(CT, C8).T)  # (128, 8)
    gbv = np.ascontiguousarray((gamma * b_v).reshape(CT, C8).T)
    b_q_col = b_q.reshape(C8, 1).astype(bf)

    in_maps = []
    for i in range(N_CORES):
        sl = slice(i * KSH, (i + 1) * KSH)
        in_maps.append(
            {
                "text_t": pretile(np.ascontiguousarray(text[:, sl].T).astype(f8)),
                "g_wt": pretile((np.ascontiguousarray(G_w[:, sl].T) * 256.0).astype(f8)),
                "xs": np.ascontiguousarray(
                    x[i * BL : (i + 1) * BL]
                    .reshape(BL, CT, 128, N)
                    .transpose(0, 2, 1, 3)
                    .reshape(BL, 128, CT * N)
                ).astype(bf),
                "w_vt": w_vt,
                "w_kt": w_kt,
                "w_q": w_q,
                "lrow_bf": l.astype(bf),
                "lw": lw.astype(bf),
                "eye32": np.eye(32, dtype=np.float32).astype(bf),
                "g_b": g_b_t,
                "b_q": b_q_col,
                "gbv": gbv,
            }
        )
    meta = {
        "gamma": gamma,
        "skip_gb": not np.any(G_b),
        "skip_bq": not np.any(b_q),
        "skip_bv": not np.any(b_v),
    }
    return in_maps, meta


def _run(inputs, trace=False, repeat=1):
    in_maps, meta = _prep_inputs(inputs)
    nc = build(meta["gamma"], meta["skip_gb"], meta["skip_bq"], meta["skip_bv"], repeat=repeat)
    res = run_bass_kernel_spmd(nc, in_maps, core_ids=list(range(N_CORES)), trace=trace)
    outs = [
        res.results[i]["out"]
        .astype(np.float32)
        .reshape(BL, 128, CT, N)
        .transpose(0, 2, 1, 3)
        .reshape(BL, C, N)
        for i in range(N_CORES)
    ]
    full = np.concatenate(outs, axis=0).reshape(B, C, H, W)
    return full, res


def kernel(**inputs) -> np.ndarray:
    full, _ = _run(inputs, trace=False)
    return full


if __name__ == "__main__":
    import reference

    inputs = {k: np.asarray(v) for k, v in reference.setup_inputs().items()}
    got = kernel(**inputs)
    print("out shape:", got.shape, got.dtype)
